# revision 16
# baseline (speedup 1.0000x reference)
"""GAT (2-layer, 8-head) Bass kernel for 8 Trainium2 NeuronCores.

Strategy (replicated-h, single collective):
  - Core d owns 512 rows (nodes) i in [512d, 512d+512).
  - Layer-1: EVERY core computes h_k = x @ W_k for ALL 4096 nodes and all
    8 heads locally via fp8 DoubleRow matmuls (2 k-subtiles per pass, 2x
    PE rate); s2 = h @ a2 rides the same matmul as 4 extra columns per
    half.  This removes all four layer-1 AllGathers, whose first-start
    floor (~80us from kernel start, regardless of trigger time) made the
    gather pipeline idle every engine for ~50us.
  - Payload stationaries [v_j*h_j | v_j] are built locally per (head,
    j-block) from the staged h via one tensor_scalar each; attention per
    head is the separable-exponential decomposition
        exp(leakyrelu(s1_i + s2_j)) = u_i * v_j * max(w_i * z_j, 1)
    with u = exp(.2 s1) (cancels in softmax), v = exp(.2 s2),
    w = exp(.8 s1), z = exp(.8 s2).  The unnormalized masked score matrix
    (transposed, [j,i]-layout) U[j,i] = max(Wb[j,i]*z_j, 1) * mask[j,i]
    is built with one RELU (ACT, with a PE mask-matmul recovering the
    "+1") or one dual-scalar TENSOR_SCALAR (DVE) per tile, then one
    TENSOR_TENSOR applying the mask per 4-tile group; contracted on the
    PE against [v*h | v] for numerator and denominator in one PSUM pass.
  - w_i (own rows) comes from a tiny dedicated fp8 matmul
    (x_own @ (W@a1)) so Wb_k is ready ~2us in.
  - Layer-2: z = mean_k elu(h'_k) for own rows; ONE small AllGather
    shares the layer-2 payload; phase E as before.  A dummy 1-element
    AllGather issued at t~0 guarantees the CC subsystem is warm.
"""

import numpy as np
import ml_dtypes

import concourse.bass as bass
import concourse.bacc as bacc
import concourse.tile as tile
import concourse.mybir as mybir
from concourse.bass_utils import run_bass_kernel_spmd
from concourse.masks import make_identity

dt = mybir.dt
Alu = mybir.AluOpType
Act = mybir.ActivationFunctionType
DR = mybir.MatmulPerfMode.DoubleRow

NCORES = 8
N, F, NH, KH, NO = 4096, 512, 64, 8, 56
P = 128
R = N // NCORES          # rows per core = 512
IT = R // P              # i-tiles per core = 4
NB = N // P              # j-blocks = 32
GRP = 4                  # j-blocks per TT group
NACT = 2                 # ACT-produced tiles per group
C2 = NO + 2              # layer-2 payload cols
HC = 4 * NH + 4          # phase-A' matmul cols per half: 4 heads + 4 s2
f8 = ml_dtypes.float8_e4m3
bf16 = ml_dtypes.bfloat16

_CACHE: dict = {}


def _build():
    nc = bacc.Bacc("TRN2", target_bir_lowering=False, debug=False,
                   num_devices=NCORES)

    # ---- I/O -----------------------------------------------------------
    xT8_d = nc.dram_tensor("xT8", [P, 4, N], dt.float8e4,
                           kind="ExternalInput")
    xT8own_d = nc.dram_tensor("xT8own", [P, 4, R], dt.float8e4,
                              kind="ExternalInput")
    wa1p_d = nc.dram_tensor("wa1p", [P, 4, 2, 97], dt.float8e4,
                            kind="ExternalInput")
    wall8_d = nc.dram_tensor("wall8", [P, 4, 2, HC], dt.float8e4,
                             kind="ExternalInput")
    maskT_d = nc.dram_tensor("maskT", [P, NB, R], dt.bfloat16,
                             kind="ExternalInput")
    wout_d = nc.dram_tensor("wout", [NH, NO], dt.bfloat16,
                            kind="ExternalInput")
    wa12_d = nc.dram_tensor("wa12", [NH, 2], dt.bfloat16,
                            kind="ExternalInput")
    out_d = nc.dram_tensor("out", [R, NO], dt.float32, kind="ExternalOutput")

    with tile.TileContext(nc) as tc:
        _emit(nc, tc, xT8_d, xT8own_d, wa1p_d, wall8_d, maskT_d, wout_d,
              wa12_d, out_d)

    nc.compile()
    return nc


def _emit(nc, tc, xT8_d, xT8own_d, wa1p_d, wall8_d, maskT_d, wout_d,
          wa12_d, out_d):
    from contextlib import ExitStack
    ctx = ExitStack()
    with ctx:
        const = ctx.enter_context(tc.tile_pool(name="const", bufs=1))
        dram = ctx.enter_context(tc.tile_pool(name="dram", bufs=1,
                                              space="DRAM"))
        pa = ctx.enter_context(tc.tile_pool(name="pa", bufs=1, space="PSUM"))
        pb = ctx.enter_context(tc.tile_pool(name="pb", bufs=2, space="PSUM"))
        pt = ctx.enter_context(tc.tile_pool(name="pt", bufs=1, space="PSUM"))
        sp = ctx.enter_context(tc.tile_pool(name="sp", bufs=3))
        hp = ctx.enter_context(tc.tile_pool(name="hp", bufs=2))
        pp2 = ctx.enter_context(tc.tile_pool(name="pp2", bufs=4))
        tp = ctx.enter_context(tc.tile_pool(name="tp", bufs=3))
        up = ctx.enter_context(tc.tile_pool(name="up", bufs=6))
        bp = ctx.enter_context(tc.tile_pool(name="bp", bufs=1))
        cp = ctx.enter_context(tc.tile_pool(name="cp", bufs=4))
        zp = ctx.enter_context(tc.tile_pool(name="zp", bufs=1))

        # ---- CC warm-up: issued first so ncfw init (~80us, runs on its
        # own clock) overlaps phase A'/B and the layer-2 gather is warm.
        warm_in = dram.tile([1, 16], dt.bfloat16, name="warm_in")
        warm_out = dram.tile([NCORES, 16], dt.bfloat16, addr_space="Shared",
                             name="warm_out")
        nc.gpsimd.collective_compute(
            "AllGather", Alu.bypass,
            ins=[warm_in.opt()], outs=[warm_out.opt()],
            replica_groups=[list(range(NCORES))])

        # ---- resident loads ------------------------------------------
        wall8_sb = const.tile([P, 4, 2, HC], dt.float8e4)
        nc.sync.dma_start(out=wall8_sb, in_=wall8_d.ap())
        wa1p_sb = const.tile([P, 4, 2, 97], dt.float8e4)
        nc.sync.dma_start(out=wa1p_sb, in_=wa1p_d.ap())
        xT8own_sb = const.tile([P, 4, R], dt.float8e4)
        nc.sync.dma_start(out=xT8own_sb, in_=xT8own_d.ap())
        wout_sb = const.tile([NH, NO], dt.bfloat16)
        nc.sync.dma_start(out=wout_sb, in_=wout_d.ap())
        wa12_sb = const.tile([NH, 2], dt.bfloat16)
        nc.sync.dma_start(out=wa12_sb, in_=wa12_d.ap())
        # x^T for all nodes, in 8 column chunks so jb0 unblocks early;
        # mask chunks interleaved on the gpsimd queue.
        xT8_sb = const.tile([P, 4, N], dt.float8e4)
        mask_sb = const.tile([P, NB, R], dt.bfloat16)

        def _x_chunk(c):
            lo, hi = R * c, R * (c + 1)
            nc.sync.dma_start(out=xT8_sb[:, :, lo:hi],
                              in_=xT8_d.ap()[:, :, lo:hi])

        def _mask_chunk(c):
            lo, hi = 4 * c, 4 * (c + 1)
            nc.gpsimd.dma_start(out=mask_sb[:, lo:hi, :],
                                in_=maskT_d.ap()[:, lo:hi, :])

        _x_chunk(0)
        _x_chunk(1)
        _mask_chunk(0)
        for c in range(2, 8):
            _x_chunk(c)
            _mask_chunk(c - 1)
        _mask_chunk(7)

        idb = const.tile([P, P], dt.bfloat16)
        make_identity(nc, idb)
        idf = const.tile([P, P], dt.float32)
        make_identity(nc, idf)
        neg1 = const.tile([P, 1], dt.float32)
        nc.vector.memset(neg1, -1.0)
        ln8 = const.tile([P, 1], dt.float32)
        nc.vector.memset(ln8, float(np.log(0.125)))
        ones1 = const.tile([1, P], dt.bfloat16)
        nc.vector.memset(ones1, 1.0)


        # persistent layer-1 tensors; each 66-col block per head is
        # [s2 | h(64) | 1], so [h | 1] (cols 1:66) is the attention
        # stationary with the ones column giving the denominator.
        staging5 = zp.tile([P, NB, 2, 4, 66], dt.bfloat16, name="staging5")
        Z1_all = zp.tile([P, NB, 2, 4], dt.float32, name="Z1_all")
        V_all = zp.tile([P, NB, 2, 4], dt.float32, name="V_all")
        u_all = zp.tile([P, IT, KH, NH], dt.float32)   # h'_k per head
        nc.vector.memset(staging5[:, :, :, :, 65], 1.0)

        ag2_in = dram.tile([P, IT, C2], dt.bfloat16)
        ag2_out = dram.tile([NCORES * P, IT, C2], dt.bfloat16,
                            addr_space="Shared")

        # ---- w rows for own i (tiny fp8 matmuls, ready ~2us); s1 rows
        # packed at partition stride 32 so the ACT exp reads are
        # base-partition aligned.
        wrows = []
        for t in range(2):
            ps_s4 = pa.tile([P, R], dt.float32, tag="wb", bufs=1,
                            name=f"ps_s4{t}")
            for fb in range(4):
                nc.tensor.matmul(ps_s4[0:97, :], wa1p_sb[:, fb, t, :],
                                 xT8own_sb[:, fb, :],
                                 start=(fb == 0), stop=(fb == 3))
            for j in range(4):
                wr = sp.tile([1, R], dt.bfloat16, tag="wrow8", bufs=8,
                             name=f"wrow{4 * t + j}")
                nc.scalar.activation(wr, ps_s4[32 * j:32 * j + 1, :],
                                     Act.Exp, scale=0.8)
                wrows.append(wr)
        Wb_k = []
        for k in range(KH):
            ps_wb = pa.tile([P, R], dt.float32, tag="wb", bufs=1)
            nc.tensor.matmul(ps_wb, ones1, wrows[k],
                             start=True, stop=True)
            Wb = sp.tile([P, R], dt.bfloat16, tag="Wb", bufs=8,
                         name=f"Wb{k}")
            nc.scalar.activation(Wb, ps_wb, Act.Copy)
            Wb_k.append(Wb)

        # ================= phase A': replicated h + payload ===========
        for jb in range(NB):
            js = slice(jb * P, (jb + 1) * P)
            for hh in range(2):
                ps_h = pa.tile([P, HC], dt.float32, tag="hn", bufs=2,
                               name=f"ps_h{hh}")
                for pr in range(2):
                    nc.tensor.matmul(ps_h,
                                     xT8_sb[:, 2 * pr:2 * pr + 2, js],
                                     wall8_sb[:, 2 * pr:2 * pr + 2, hh, :],
                                     start=(pr == 0), stop=(pr == 1),
                                     perf_mode=DR)
                nc.scalar.activation(
                    staging5[:, jb, hh, :, 0:65],
                    ps_h[:, :].rearrange("p (q c) -> p q c", c=65),
                    Act.Copy)
            if jb % 4 == 3:
                g = slice(jb - 3, jb + 1)
                nc.scalar.activation(Z1_all[:, g, :, :],
                                     staging5[:, g, :, :, 0],
                                     Act.Exp, scale=1.0)
                nc.scalar.activation(V_all[:, g, :, :],
                                     staging5[:, g, :, :, 0],
                                     Act.Exp, scale=0.2)

        # ================= phase B: per-head attention ================
        def attn_units(Wb, ps_nm, triples, grp, nact, first, last, ttag):
            """triples: (stationary_ap, z_scalar_ap, jb) in order; grp
            consecutive-jb blocks share one mask TT; first nact of each
            group are ACT-produced (relu(z*w-1), '+1' recovered by an
            extra mask matmul)."""
            n = len(triples)
            for g0 in range(0, n, grp):
                seg = triples[g0:g0 + grp]
                Tg = tp.tile([P, grp, R], dt.bfloat16, tag=ttag)
                for q, (stat, zsc, jb) in enumerate(seg):
                    if isinstance(zsc, tuple):
                        nc.vector.tensor_scalar(Tg[:, q, :], Wb, zsc[0],
                                                zsc[1], Alu.mult, Alu.max)
                    elif q < nact:
                        nc.scalar.activation(Tg[:, q, :], Wb, Act.Relu,
                                             bias=neg1[:, 0:1], scale=zsc)
                    else:
                        nc.vector.tensor_scalar(Tg[:, q, :], Wb, zsc, 1.0,
                                                Alu.mult, Alu.max)
                jb0 = seg[0][2]
                Ug = up.tile([P, grp, R], dt.bfloat16, tag=ttag + "u")
                nc.vector.tensor_tensor(Ug, Tg,
                                        mask_sb[:, jb0:jb0 + grp, :],
                                        Alu.mult)
                for q, (stat, zsc, jb) in enumerate(seg):
                    st = first and g0 == 0 and q == 0
                    sp_ = last and g0 + grp >= n and q == len(seg) - 1
                    nc.tensor.matmul(ps_nm, stat, Ug[:, q, :],
                                     start=st, stop=sp_)
                    if q < nact:
                        nc.tensor.matmul(ps_nm, stat, mask_sb[:, jb, :],
                                         start=False, stop=False)

        def finish_unit(ps_nm, ncols):
            nmf = sp.tile([ncols, R], dt.float32, tag="nmf")
            nc.any.tensor_copy(nmf, ps_nm)
            ps_t = pt.tile([P, IT, ncols], dt.float32, tag="tr")
            for isl in range(IT):
                sl = slice(isl * P, (isl + 1) * P)
                nc.tensor.transpose(ps_t[:, isl, :], nmf[:, sl],
                                    idf[0:ncols, 0:ncols])
            return ps_t

        ps_misc = pa.tile([P, IT, NH], dt.float32, tag="sall")
        ps_zsum = ps_misc
        for k in range(KH):
            ps_nm = pb.tile([NH + 1, R], dt.float32, tag="nm")
            triples = [(staging5[:, jb, k // 4, k % 4, 1:66],
                        (Z1_all[:, jb, k // 4, k % 4:k % 4 + 1],
                         V_all[:, jb, k // 4, k % 4:k % 4 + 1]), jb)
                       for jb in range(NB)]
            attn_units(Wb_k[k], ps_nm, triples, GRP, 0, True, True, "T")
            ps_t = finish_unit(ps_nm, NH + 1)
            rc4 = sp.tile([P, IT], dt.float32, tag="rc")
            nc.vector.reciprocal(rc4, ps_t[:, :, NH])
            for isl in range(IT):
                nc.scalar.activation(u_all[:, isl, k, :],
                                     ps_t[:, isl, 0:NH], Act.Copy,
                                     scale=rc4[:, isl:isl + 1])
            rn = cp.tile([P, IT, NH], dt.float32, tag="rn")
            nc.scalar.activation(rn, u_all[:, :, k, :], Act.Relu,
                                 scale=-1.0)
            Bpp = cp.tile([P, IT, NH], dt.bfloat16, tag="Bp")
            nc.scalar.activation(Bpp, rn, Act.Exp, bias=ln8[:, 0:1],
                                 scale=-1.0)
            Dpp = cp.tile([P, IT, NH], dt.bfloat16, tag="Dp")
            nc.scalar.activation(Dpp, u_all[:, :, k, :], Act.Relu,
                                 scale=0.125)
            nc.tensor.matmul(ps_zsum, idb, Dpp,
                             start=(k == 0), stop=False)
            nc.tensor.matmul(ps_zsum, idb, Bpp,
                             start=False, stop=(k == KH - 1))

        # ================= phase D: layer-2 h2 + payload (2 halves) =====
        zbf = zp.tile([P, IT, NH], dt.bfloat16)
        ps_zT = pa.tile([NH, R], dt.bfloat16, tag="wb", bufs=1)
        zT = hp.tile([NH, R], dt.bfloat16, tag="hT_s")
        h2T = hp.tile([NO, R], dt.bfloat16, tag="h2T_s")
        s2row = sp.tile([2, R], dt.bfloat16, tag="srow_s", bufs=1)
        w2row = sp.tile([1, R], dt.bfloat16, tag="w2row", bufs=1)
        ps_wb2 = pa.tile([P, R], dt.float32, tag="hn", bufs=2)
        Wb2 = sp.tile([P, R], dt.bfloat16, tag="Wb2", bufs=1)
        ps_h2n = pa.tile([P, IT, C2], dt.bfloat16, tag="hn", bufs=2)
        v2col = sp.tile([P, IT], dt.float32, tag="v2col", bufs=1)
        for h in range(2):
            hs = slice(h * 256, (h + 1) * 256)
            nc.vector.tensor_scalar(zbf[:, 2 * h:2 * h + 2, :],
                                    ps_zsum[:, 2 * h:2 * h + 2, :],
                                    -1.0, None, Alu.add)
            for isl in (2 * h, 2 * h + 1):
                sl = slice(isl * P, (isl + 1) * P)
                nc.tensor.transpose(ps_zT[:, sl], zbf[:, isl, :], idb)
            nc.vector.tensor_copy(zT[:, hs], ps_zT[:, hs])
            ps_h2T = pa.tile([NO, 256], dt.float32, tag="srow",
                             name=f"ps_h2T{h}")
            nc.tensor.matmul(ps_h2T, wout_sb, zT[:, hs],
                             start=True, stop=True)
            nc.any.tensor_copy(h2T[:, hs], ps_h2T)
            ps_s12 = pa.tile([2, 256], dt.float32, tag="srow",
                             name=f"ps_s12{h}")
            nc.tensor.matmul(ps_s12, wa12_sb, zT[:, hs],
                             start=True, stop=True)
            nc.any.tensor_copy(s2row[:, hs], ps_s12)
            nc.scalar.activation(w2row[:, hs], s2row[0:1, hs], Act.Exp,
                                 scale=0.8)
            nc.tensor.matmul(ps_wb2[:, hs], ones1, w2row[:, hs],
                             start=True, stop=True)
            nc.vector.tensor_copy(Wb2[:, hs], ps_wb2[:, hs])
            pay2 = pp2.tile([P, 2, C2], dt.bfloat16, tag="pay2")
            for ii, isl in enumerate((2 * h, 2 * h + 1)):
                sl = slice(isl * P, (isl + 1) * P)
                nc.tensor.transpose(ps_h2n[:, isl, 0:NO], h2T[:, sl],
                                    idb[0:NO, 0:NO])
                nc.tensor.transpose(ps_h2n[:, isl, NO:NO + 2], s2row[:, sl],
                                    idb[0:2, 0:2])
            nc.scalar.activation(v2col[:, 2 * h:2 * h + 2],
                                 ps_h2n[:, 2 * h:2 * h + 2, NO + 1],
                                 Act.Exp, scale=0.2)
            for ii, isl in enumerate((2 * h, 2 * h + 1)):
                nc.vector.tensor_scalar(pay2[:, ii, 0:NO],
                                        ps_h2n[:, isl, 0:NO],
                                        v2col[:, isl:isl + 1], None,
                                        Alu.mult)
            nc.vector.tensor_copy(pay2[:, :, NO],
                                  v2col[:, 2 * h:2 * h + 2])
            nc.scalar.activation(pay2[:, :, NO + 1],
                                 ps_h2n[:, 2 * h:2 * h + 2, NO + 1],
                                 Act.Exp, scale=0.8)
            nc.sync.dma_start(out=ag2_in[:, 2 * h:2 * h + 2, :], in_=pay2)
        nc.gpsimd.collective_compute(
            "AllGather", Alu.bypass,
            ins=[ag2_in.opt()], outs=[ag2_out.opt()],
            replica_groups=[list(range(NCORES))])

        # ================= phase E: attention-2 + softmax ===============
        zf2 = sp.tile([P, NCORES, IT], dt.float32, tag="zf")
        ps_nm2 = pb.tile([NO + 1, R], dt.float32, tag="nm")
        hsb2 = bp.tile([P, NCORES, IT, C2], dt.bfloat16, name="hsb2")
        nc.sync.dma_start(
            out=hsb2,
            in_=ag2_out.rearrange("(core p) isl c -> p core isl c", p=P))
        nc.vector.tensor_copy(zf2, hsb2[:, :, :, NO + 1])
        triples = [(hsb2[:, jb // IT, jb % IT, 0:NO + 1],
                    zf2[:, jb // IT, jb % IT:jb % IT + 1], jb)
                   for jb in range(NB)]
        attn_units(Wb2, ps_nm2, triples, GRP, 2, True, True, "T")
        ps_t2 = finish_unit(ps_nm2, NO + 1)
        for isl in range(IT):
            rc = sp.tile([P, 1], dt.float32, tag="rc")
            nc.vector.reciprocal(rc, ps_t2[:, isl, NO:NO + 1])
            ue = cp.tile([P, NO], dt.float32, tag="ue")
            nc.vector.tensor_scalar(ue, ps_t2[:, isl, 0:NO], rc, None,
                                    Alu.mult)
            e2 = cp.tile([P, NO], dt.float32, tag="e2")
            nc.scalar.activation(e2, ue, Act.Exp)
            t1 = cp.tile([P, NO], dt.float32, tag="t1")
            nc.vector.tensor_scalar(t1, e2, 1.0, -1.0, Alu.min, Alu.add)
            el = cp.tile([P, NO], dt.float32, tag="el")
            nc.vector.scalar_tensor_tensor(el, ue, 0.0, t1, Alu.max, Alu.add)
            # |elu(h2')| is O(1): plain exp is overflow-safe, and the
            # max-subtraction cancels exactly in the softmax ratio
            ex = cp.tile([P, NO], dt.float32, tag="ex")
            sm = sp.tile([P, 1], dt.float32, tag="sm")
            nc.scalar.activation(ex, el, Act.Exp, accum_out=sm)
            rc2 = sp.tile([P, 1], dt.float32, tag="rc2")
            nc.vector.reciprocal(rc2, sm)
            oo = cp.tile([P, NO], dt.float32, tag="oo")
            nc.vector.tensor_scalar(oo, ex, rc2, None, Alu.mult)
            nc.sync.dma_start(out=out_d.ap()[isl * P:(isl + 1) * P, :],
                              in_=oo)


def _prep_inputs(x, adj, Ws, As, W_out, a_out):
    x64 = np.asarray(x, np.float64)
    adj_np = np.asarray(adj)
    mask_full = adj_np > 0
    Ws64 = np.asarray(Ws, np.float64)               # [8, 512, 64]
    As64 = np.asarray(As, np.float64)[:, :, 0]      # [8, 128]

    xT8 = np.ascontiguousarray(
        x64.T.reshape(4, P, N).transpose(1, 0, 2)).astype(f8)

    # wall8: per half hh, cols [h_{4 heads} (256) | s2_{4 heads} (4)]
    wall = np.zeros((F, 2, HC), np.float64)
    wa1 = np.zeros((F, 2, 97), np.float64)
    for k in range(KH):
        hh, q = k // 4, k % 4
        wall[:, hh, 65 * q] = Ws64[k] @ As64[k, NH:]
        wall[:, hh, 65 * q + 1:65 * q + 65] = Ws64[k]
        wa1[:, hh, 32 * q] = Ws64[k] @ As64[k, :NH]
    wall8 = np.ascontiguousarray(
        wall.reshape(4, P, 2, HC).transpose(1, 0, 2, 3)).astype(f8)
    wa1p = np.ascontiguousarray(
        wa1.reshape(4, P, 2, 97).transpose(1, 0, 2, 3)).astype(f8)

    wout = np.asarray(W_out, np.float32).astype(bf16)
    Wo64 = np.asarray(W_out, np.float64)
    ao = np.asarray(a_out, np.float64)[:, 0]
    wa12 = np.ascontiguousarray(
        np.stack([Wo64 @ ao[:NO], Wo64 @ ao[NO:]], axis=-1)).astype(bf16)

    in_maps = []
    for d in range(NCORES):
        rows = slice(R * d, R * (d + 1))
        xT8own = np.ascontiguousarray(xT8[:, :, rows])
        maskT = np.ascontiguousarray(
            mask_full[rows].T.astype(bf16).reshape(NB, P, R)
            .transpose(1, 0, 2))
        in_maps.append({
            "xT8": xT8, "xT8own": xT8own, "wa1p": wa1p, "wall8": wall8,
            "maskT": maskT, "wout": wout, "wa12": wa12,
        })
    return in_maps


def kernel(x, adj, Ws, As, W_out, a_out, trace=False):
    if "nc" not in _CACHE:
        _CACHE["nc"] = _build()
    nc = _CACHE["nc"]
    in_maps = _prep_inputs(x, adj, Ws, As, W_out, a_out)
    res = run_bass_kernel_spmd(nc, in_maps, list(range(NCORES)), trace=trace)
    out = np.concatenate([res.results[d]["out"] for d in range(NCORES)],
                         axis=0).astype(np.float32)
    if trace:
        kernel.last_exec_time_ns = res.exec_time_ns
    return out


# revision 17
# speedup vs baseline: 1.2323x; 1.2323x over previous
"""GAT (2-layer, 8-head) Bass kernel for 8 Trainium2 NeuronCores.

Strategy (replicated-h, single collective):
  - Core d owns 512 rows (nodes) i in [512d, 512d+512).
  - Layer-1: EVERY core computes h_k = x @ W_k for ALL 4096 nodes and all
    8 heads locally via fp8 DoubleRow matmuls (2 k-subtiles per pass, 2x
    PE rate); s2 = h @ a2 rides the same matmul as 4 extra columns per
    half.  This removes all four layer-1 AllGathers, whose first-start
    floor (~80us from kernel start, regardless of trigger time) made the
    gather pipeline idle every engine for ~50us.
  - Payload stationaries [v_j*h_j | v_j] are built locally per (head,
    j-block) from the staged h via one tensor_scalar each; attention per
    head is the separable-exponential decomposition
        exp(leakyrelu(s1_i + s2_j)) = u_i * v_j * max(w_i * z_j, 1)
    with u = exp(.2 s1) (cancels in softmax), v = exp(.2 s2),
    w = exp(.8 s1), z = exp(.8 s2).  The unnormalized masked score matrix
    (transposed, [j,i]-layout) U[j,i] = max(Wb[j,i]*z_j, 1) * mask[j,i]
    is built with one RELU (ACT, with a PE mask-matmul recovering the
    "+1") or one dual-scalar TENSOR_SCALAR (DVE) per tile, then one
    TENSOR_TENSOR applying the mask per 4-tile group; contracted on the
    PE against [v*h | v] for numerator and denominator in one PSUM pass.
  - w_i (own rows) comes from a tiny dedicated fp8 matmul
    (x_own @ (W@a1)) so Wb_k is ready ~2us in.
  - Layer-2: z = mean_k elu(h'_k) for own rows; ONE small AllGather
    shares the layer-2 payload; phase E as before.  A dummy 1-element
    AllGather issued at t~0 guarantees the CC subsystem is warm.
"""

import numpy as np
import ml_dtypes

import concourse.bass as bass
import concourse.bacc as bacc
import concourse.tile as tile
import concourse.mybir as mybir
from concourse.bass_utils import run_bass_kernel_spmd
from concourse.masks import make_identity

dt = mybir.dt
Alu = mybir.AluOpType
Act = mybir.ActivationFunctionType
DR = mybir.MatmulPerfMode.DoubleRow

NCORES = 8
N, F, NH, KH, NO = 4096, 512, 64, 8, 56
P = 128
R = N // NCORES          # rows per core = 512
IT = R // P              # i-tiles per core = 4
NB = N // P              # j-blocks = 32
GRP = 4                  # j-blocks per TT group
NACT = 2                 # ACT-produced tiles per group
C2 = NO + 2              # layer-2 payload cols
HC = 4 * NH + 4          # phase-A' matmul cols per half: 4 heads + 4 s2
f8 = ml_dtypes.float8_e4m3
bf16 = ml_dtypes.bfloat16

_CACHE: dict = {}


def _build():
    nc = bacc.Bacc("TRN2", target_bir_lowering=False, debug=False,
                   num_devices=NCORES)

    # ---- I/O -----------------------------------------------------------
    xT8_d = nc.dram_tensor("xT8", [P, 4, N], dt.float8e4,
                           kind="ExternalInput")
    xT8own_d = nc.dram_tensor("xT8own", [P, 4, R], dt.float8e4,
                              kind="ExternalInput")
    wa1p_d = nc.dram_tensor("wa1p", [P, 4, 2, 97], dt.float8e4,
                            kind="ExternalInput")
    wall8_d = nc.dram_tensor("wall8", [P, 4, 2, HC], dt.float8e4,
                             kind="ExternalInput")
    maskT_d = nc.dram_tensor("maskT", [P, NB, R], dt.bfloat16,
                             kind="ExternalInput")
    wout_d = nc.dram_tensor("wout", [NH, NO], dt.bfloat16,
                            kind="ExternalInput")
    wa12_d = nc.dram_tensor("wa12", [NH, 2], dt.bfloat16,
                            kind="ExternalInput")
    out_d = nc.dram_tensor("out", [R, NO], dt.float32, kind="ExternalOutput")

    with tile.TileContext(nc) as tc:
        _emit(nc, tc, xT8_d, xT8own_d, wa1p_d, wall8_d, maskT_d, wout_d,
              wa12_d, out_d)

    nc.compile()
    return nc


def _emit(nc, tc, xT8_d, xT8own_d, wa1p_d, wall8_d, maskT_d, wout_d,
          wa12_d, out_d):
    from contextlib import ExitStack
    ctx = ExitStack()
    with ctx:
        const = ctx.enter_context(tc.tile_pool(name="const", bufs=1))
        dram = ctx.enter_context(tc.tile_pool(name="dram", bufs=1,
                                              space="DRAM"))
        pa = ctx.enter_context(tc.tile_pool(name="pa", bufs=1, space="PSUM"))
        pb = ctx.enter_context(tc.tile_pool(name="pb", bufs=2, space="PSUM"))
        pt = ctx.enter_context(tc.tile_pool(name="pt", bufs=1, space="PSUM"))
        sp = ctx.enter_context(tc.tile_pool(name="sp", bufs=3))
        hp = ctx.enter_context(tc.tile_pool(name="hp", bufs=2))
        pp2 = ctx.enter_context(tc.tile_pool(name="pp2", bufs=4))
        tp = ctx.enter_context(tc.tile_pool(name="tp", bufs=3))
        up = ctx.enter_context(tc.tile_pool(name="up", bufs=4))
        bp = ctx.enter_context(tc.tile_pool(name="bp", bufs=1))
        cp = ctx.enter_context(tc.tile_pool(name="cp", bufs=4))
        zp = ctx.enter_context(tc.tile_pool(name="zp", bufs=1))

        # ---- CC warm-up: issued first so ncfw init (~80us, runs on its
        # own clock) overlaps phase A'/B and the layer-2 gather is warm.
        warm_in = dram.tile([1, 16], dt.bfloat16, name="warm_in")
        warm_out = dram.tile([NCORES, 16], dt.bfloat16, addr_space="Shared",
                             name="warm_out")
        nc.gpsimd.collective_compute(
            "AllGather", Alu.bypass,
            ins=[warm_in.opt()], outs=[warm_out.opt()],
            replica_groups=[list(range(NCORES))])

        # ---- resident loads ------------------------------------------
        wall8_sb = const.tile([P, 4, 2, HC], dt.float8e4)
        nc.sync.dma_start(out=wall8_sb, in_=wall8_d.ap())
        wa1p_sb = const.tile([P, 4, 2, 97], dt.float8e4)
        nc.sync.dma_start(out=wa1p_sb, in_=wa1p_d.ap())
        xT8own_sb = const.tile([P, 4, R], dt.float8e4)
        nc.sync.dma_start(out=xT8own_sb, in_=xT8own_d.ap())
        wout_sb = const.tile([NH, NO], dt.bfloat16)
        nc.sync.dma_start(out=wout_sb, in_=wout_d.ap())
        wa12_sb = const.tile([NH, 2], dt.bfloat16)
        nc.sync.dma_start(out=wa12_sb, in_=wa12_d.ap())
        # x^T for all nodes, in 8 column chunks so jb0 unblocks early;
        # mask chunks interleaved on the gpsimd queue.
        xT8_sb = const.tile([P, 4, N], dt.float8e4)
        mask_sb = const.tile([P, NB, R], dt.bfloat16)

        def _x_chunk(c):
            lo, hi = R * c, R * (c + 1)
            nc.sync.dma_start(out=xT8_sb[:, :, lo:hi],
                              in_=xT8_d.ap()[:, :, lo:hi])

        def _mask_chunk(c):
            lo, hi = 4 * c, 4 * (c + 1)
            nc.gpsimd.dma_start(out=mask_sb[:, lo:hi, :],
                                in_=maskT_d.ap()[:, lo:hi, :])

        _x_chunk(0)
        _x_chunk(1)
        _mask_chunk(0)
        for c in range(2, 8):
            _x_chunk(c)
            _mask_chunk(c - 1)
        _mask_chunk(7)

        idb = const.tile([P, P], dt.bfloat16)
        make_identity(nc, idb)
        idf = const.tile([P, P], dt.float32)
        make_identity(nc, idf)
        neg1 = const.tile([P, 1], dt.float32)
        nc.vector.memset(neg1, -1.0)
        ln8 = const.tile([P, 1], dt.float32)
        nc.vector.memset(ln8, float(np.log(0.125)))
        ones1 = const.tile([1, P], dt.bfloat16)
        nc.vector.memset(ones1, 1.0)

        # persistent layer-1 tensors
        staging = zp.tile([P, NB, 2, HC], dt.bfloat16, name="staging")
        pay_v = zp.tile([P, NB, 2, 4, NH + 1], dt.bfloat16, name="pay_v")
        z_all = zp.tile([P, NB, 2, 4], dt.float32, name="z_all")
        v_all = zp.tile([P, NB, 2, 4], dt.float32, name="v_all")
        u_all = zp.tile([P, IT, KH, NH], dt.float32)   # h'_k per head

        ag2_in = dram.tile([P, IT, C2], dt.bfloat16)
        ag2_out = dram.tile([NCORES * P, IT, C2], dt.bfloat16,
                            addr_space="Shared")

        # ---- w rows for own i (tiny fp8 matmuls, ready ~2us); s1 rows
        # packed at partition stride 32 so the ACT exp reads are
        # base-partition aligned.
        wrows = []
        for t in range(2):
            ps_s4 = pa.tile([P, R], dt.float32, tag="wb", bufs=1,
                            name=f"ps_s4{t}")
            for fb in range(4):
                nc.tensor.matmul(ps_s4[0:97, :], wa1p_sb[:, fb, t, :],
                                 xT8own_sb[:, fb, :],
                                 start=(fb == 0), stop=(fb == 3))
            for j in range(4):
                wr = sp.tile([1, R], dt.bfloat16, tag="wrow8", bufs=8,
                             name=f"wrow{4 * t + j}")
                nc.scalar.activation(wr, ps_s4[32 * j:32 * j + 1, :],
                                     Act.Exp, scale=0.8)
                wrows.append(wr)
        Wb_k = []
        for k in range(KH):
            ps_wb = pa.tile([P, R], dt.float32, tag="wb", bufs=1)
            nc.tensor.matmul(ps_wb, ones1, wrows[k],
                             start=True, stop=True)
            Wb = sp.tile([P, R], dt.bfloat16, tag="Wb", bufs=8,
                         name=f"Wb{k}")
            nc.vector.tensor_copy(Wb, ps_wb)
            Wb_k.append(Wb)

        # ================= phase A': replicated h + payload ===========
        for jb in range(NB):
            js = slice(jb * P, (jb + 1) * P)
            for hh in range(2):
                ps_h = pa.tile([P, HC], dt.float32, tag="hn", bufs=2,
                               name=f"ps_h{hh}")
                for pr in range(2):
                    nc.tensor.matmul(ps_h,
                                     xT8_sb[:, 2 * pr:2 * pr + 2, js],
                                     wall8_sb[:, 2 * pr:2 * pr + 2, hh, :],
                                     start=(pr == 0), stop=(pr == 1),
                                     perf_mode=DR)
                nc.scalar.activation(staging[:, jb, hh, :], ps_h,
                                     Act.Copy)
            if jb % 4 == 3:
                g = slice(jb - 3, jb + 1)
                nc.scalar.activation(z_all[:, g, :, :],
                                     staging[:, g, :, 4 * NH:],
                                     Act.Exp, scale=0.8)
                nc.scalar.activation(v_all[:, g, :, :],
                                     staging[:, g, :, 4 * NH:],
                                     Act.Exp, scale=0.2)
                nc.vector.tensor_copy(pay_v[:, g, :, :, NH],
                                      v_all[:, g, :, :])
                for jb2 in range(jb - 3, jb + 1):
                    for k in range(KH):
                        hh, q = k // 4, k % 4
                        nc.vector.tensor_scalar(
                            pay_v[:, jb2, hh, q, 0:NH],
                            staging[:, jb2, hh, q * NH:(q + 1) * NH],
                            v_all[:, jb2, hh, q:q + 1], None, Alu.mult)

        # ================= phase B: per-head attention ================
        def attn_units(Wb, ps_nm, triples, grp, nact, first, last, ttag):
            """triples: (stationary_ap, z_scalar_ap, jb) in order; grp
            consecutive-jb blocks share one mask TT; first nact of each
            group are ACT-produced (relu(z*w-1), '+1' recovered by an
            extra mask matmul)."""
            n = len(triples)
            for g0 in range(0, n, grp):
                seg = triples[g0:g0 + grp]
                Tg = tp.tile([P, grp, R], dt.bfloat16, tag=ttag)
                for q, (stat, zsc, jb) in enumerate(seg):
                    if q < nact:
                        nc.scalar.activation(Tg[:, q, :], Wb, Act.Relu,
                                             bias=neg1[:, 0:1], scale=zsc)
                    else:
                        nc.vector.tensor_scalar(Tg[:, q, :], Wb, zsc, 1.0,
                                                Alu.mult, Alu.max)
                jb0 = seg[0][2]
                Ug = up.tile([P, grp, R], dt.bfloat16, tag=ttag + "u")
                nc.vector.tensor_tensor(Ug, Tg,
                                        mask_sb[:, jb0:jb0 + grp, :],
                                        Alu.mult)
                for q, (stat, zsc, jb) in enumerate(seg):
                    st = first and g0 == 0 and q == 0
                    sp_ = last and g0 + grp >= n and q == len(seg) - 1
                    nc.tensor.matmul(ps_nm, stat, Ug[:, q, :],
                                     start=st, stop=sp_)
                    if q < nact:
                        nc.tensor.matmul(ps_nm, stat, mask_sb[:, jb, :],
                                         start=False, stop=False)

        def finish_unit(ps_nm, ncols):
            nmf = sp.tile([ncols, R], dt.float32, tag="nmf")
            nc.any.tensor_copy(nmf, ps_nm)
            ps_t = pt.tile([P, IT, ncols], dt.float32, tag="tr")
            for isl in range(IT):
                sl = slice(isl * P, (isl + 1) * P)
                nc.tensor.transpose(ps_t[:, isl, :], nmf[:, sl],
                                    idf[0:ncols, 0:ncols])
            return ps_t

        ps_misc = pa.tile([P, IT, NH], dt.float32, tag="sall")
        ps_zsum = ps_misc
        for k in range(KH):
            ps_nm = pb.tile([NH + 1, R], dt.float32, tag="nm")
            triples = [(pay_v[:, jb, k // 4, k % 4, :],
                        z_all[:, jb, k // 4, k % 4:k % 4 + 1], jb)
                       for jb in range(NB)]
            attn_units(Wb_k[k], ps_nm, triples, GRP, NACT, True, True, "T")
            ps_t = finish_unit(ps_nm, NH + 1)
            rc4 = sp.tile([P, IT], dt.float32, tag="rc")
            nc.vector.reciprocal(rc4, ps_t[:, :, NH])
            for isl in range(IT):
                nc.scalar.activation(u_all[:, isl, k, :],
                                     ps_t[:, isl, 0:NH], Act.Copy,
                                     scale=rc4[:, isl:isl + 1])
            rn = cp.tile([P, IT, NH], dt.float32, tag="rn")
            nc.scalar.activation(rn, u_all[:, :, k, :], Act.Relu,
                                 scale=-1.0)
            Bpp = cp.tile([P, IT, NH], dt.bfloat16, tag="Bp")
            nc.scalar.activation(Bpp, rn, Act.Exp, bias=ln8[:, 0:1],
                                 scale=-1.0)
            Dpp = cp.tile([P, IT, NH], dt.bfloat16, tag="Dp")
            nc.vector.tensor_scalar(Dpp, u_all[:, :, k, :], 0.125, 0.0,
                                    Alu.mult, Alu.max)
            nc.tensor.matmul(ps_zsum, idb, Dpp,
                             start=(k == 0), stop=False)
            nc.tensor.matmul(ps_zsum, idb, Bpp,
                             start=False, stop=(k == KH - 1))

        # ================= phase D: layer-2 h2 + payload (2 halves) =====
        zbf = zp.tile([P, IT, NH], dt.bfloat16)
        ps_zT = pa.tile([NH, R], dt.bfloat16, tag="wb", bufs=1)
        zT = hp.tile([NH, R], dt.bfloat16, tag="hT_s")
        h2T = hp.tile([NO, R], dt.bfloat16, tag="h2T_s")
        s2row = sp.tile([2, R], dt.bfloat16, tag="srow_s", bufs=1)
        w2row = sp.tile([1, R], dt.bfloat16, tag="w2row", bufs=1)
        ps_wb2 = pa.tile([P, R], dt.float32, tag="hn", bufs=2)
        Wb2 = sp.tile([P, R], dt.bfloat16, tag="Wb2", bufs=1)
        ps_h2n = pa.tile([P, IT, C2], dt.bfloat16, tag="hn", bufs=2)
        v2col = sp.tile([P, IT], dt.float32, tag="v2col", bufs=1)
        for h in range(2):
            hs = slice(h * 256, (h + 1) * 256)
            nc.vector.tensor_scalar(zbf[:, 2 * h:2 * h + 2, :],
                                    ps_zsum[:, 2 * h:2 * h + 2, :],
                                    -1.0, None, Alu.add)
            for isl in (2 * h, 2 * h + 1):
                sl = slice(isl * P, (isl + 1) * P)
                nc.tensor.transpose(ps_zT[:, sl], zbf[:, isl, :], idb)
            nc.vector.tensor_copy(zT[:, hs], ps_zT[:, hs])
            ps_h2T = pa.tile([NO, 256], dt.float32, tag="srow",
                             name=f"ps_h2T{h}")
            nc.tensor.matmul(ps_h2T, wout_sb, zT[:, hs],
                             start=True, stop=True)
            nc.any.tensor_copy(h2T[:, hs], ps_h2T)
            ps_s12 = pa.tile([2, 256], dt.float32, tag="srow",
                             name=f"ps_s12{h}")
            nc.tensor.matmul(ps_s12, wa12_sb, zT[:, hs],
                             start=True, stop=True)
            nc.any.tensor_copy(s2row[:, hs], ps_s12)
            nc.scalar.activation(w2row[:, hs], s2row[0:1, hs], Act.Exp,
                                 scale=0.8)
            nc.tensor.matmul(ps_wb2[:, hs], ones1, w2row[:, hs],
                             start=True, stop=True)
            nc.vector.tensor_copy(Wb2[:, hs], ps_wb2[:, hs])
            pay2 = pp2.tile([P, 2, C2], dt.bfloat16, tag="pay2")
            for ii, isl in enumerate((2 * h, 2 * h + 1)):
                sl = slice(isl * P, (isl + 1) * P)
                nc.tensor.transpose(ps_h2n[:, isl, 0:NO], h2T[:, sl],
                                    idb[0:NO, 0:NO])
                nc.tensor.transpose(ps_h2n[:, isl, NO:NO + 2], s2row[:, sl],
                                    idb[0:2, 0:2])
            nc.scalar.activation(v2col[:, 2 * h:2 * h + 2],
                                 ps_h2n[:, 2 * h:2 * h + 2, NO + 1],
                                 Act.Exp, scale=0.2)
            for ii, isl in enumerate((2 * h, 2 * h + 1)):
                nc.vector.tensor_scalar(pay2[:, ii, 0:NO],
                                        ps_h2n[:, isl, 0:NO],
                                        v2col[:, isl:isl + 1], None,
                                        Alu.mult)
            nc.vector.tensor_copy(pay2[:, :, NO],
                                  v2col[:, 2 * h:2 * h + 2])
            nc.scalar.activation(pay2[:, :, NO + 1],
                                 ps_h2n[:, 2 * h:2 * h + 2, NO + 1],
                                 Act.Exp, scale=0.8)
            nc.sync.dma_start(out=ag2_in[:, 2 * h:2 * h + 2, :], in_=pay2)
        nc.gpsimd.collective_compute(
            "AllGather", Alu.bypass,
            ins=[ag2_in.opt()], outs=[ag2_out.opt()],
            replica_groups=[list(range(NCORES))])

        # ================= phase E: attention-2 + softmax ===============
        zf2 = sp.tile([P, NCORES, IT], dt.float32, tag="zf")
        ps_nm2 = pb.tile([NO + 1, R], dt.float32, tag="nm")
        hsb2 = bp.tile([P, NCORES, IT, C2], dt.bfloat16, name="hsb2")
        nc.sync.dma_start(
            out=hsb2,
            in_=ag2_out.rearrange("(core p) isl c -> p core isl c", p=P))
        nc.vector.tensor_copy(zf2, hsb2[:, :, :, NO + 1])
        triples = [(hsb2[:, jb // IT, jb % IT, 0:NO + 1],
                    zf2[:, jb // IT, jb % IT:jb % IT + 1], jb)
                   for jb in range(NB)]
        attn_units(Wb2, ps_nm2, triples, GRP, 2, True, True, "T")
        ps_t2 = finish_unit(ps_nm2, NO + 1)
        for isl in range(IT):
            rc = sp.tile([P, 1], dt.float32, tag="rc")
            nc.vector.reciprocal(rc, ps_t2[:, isl, NO:NO + 1])
            ue = cp.tile([P, NO], dt.float32, tag="ue")
            nc.vector.tensor_scalar(ue, ps_t2[:, isl, 0:NO], rc, None,
                                    Alu.mult)
            e2 = cp.tile([P, NO], dt.float32, tag="e2")
            nc.scalar.activation(e2, ue, Act.Exp)
            t1 = cp.tile([P, NO], dt.float32, tag="t1")
            nc.vector.tensor_scalar(t1, e2, 1.0, -1.0, Alu.min, Alu.add)
            el = cp.tile([P, NO], dt.float32, tag="el")
            nc.vector.scalar_tensor_tensor(el, ue, 0.0, t1, Alu.max, Alu.add)
            # |elu(h2')| is O(1): plain exp is overflow-safe, and the
            # max-subtraction cancels exactly in the softmax ratio
            ex = cp.tile([P, NO], dt.float32, tag="ex")
            sm = sp.tile([P, 1], dt.float32, tag="sm")
            nc.scalar.activation(ex, el, Act.Exp, accum_out=sm)
            rc2 = sp.tile([P, 1], dt.float32, tag="rc2")
            nc.vector.reciprocal(rc2, sm)
            oo = cp.tile([P, NO], dt.float32, tag="oo")
            nc.vector.tensor_scalar(oo, ex, rc2, None, Alu.mult)
            nc.sync.dma_start(out=out_d.ap()[isl * P:(isl + 1) * P, :],
                              in_=oo)


def _prep_inputs(x, adj, Ws, As, W_out, a_out):
    x64 = np.asarray(x, np.float64)
    adj_np = np.asarray(adj)
    mask_full = adj_np > 0
    Ws64 = np.asarray(Ws, np.float64)               # [8, 512, 64]
    As64 = np.asarray(As, np.float64)[:, :, 0]      # [8, 128]

    xT8 = np.ascontiguousarray(
        x64.T.reshape(4, P, N).transpose(1, 0, 2)).astype(f8)

    # wall8: per half hh, cols [h_{4 heads} (256) | s2_{4 heads} (4)]
    wall = np.zeros((F, 2, HC), np.float64)
    wa1 = np.zeros((F, 2, 97), np.float64)
    for k in range(KH):
        hh, q = k // 4, k % 4
        wall[:, hh, q * NH:(q + 1) * NH] = Ws64[k]
        wall[:, hh, 4 * NH + q] = Ws64[k] @ As64[k, NH:]
        wa1[:, hh, 32 * q] = Ws64[k] @ As64[k, :NH]
    wall8 = np.ascontiguousarray(
        wall.reshape(4, P, 2, HC).transpose(1, 0, 2, 3)).astype(f8)
    wa1p = np.ascontiguousarray(
        wa1.reshape(4, P, 2, 97).transpose(1, 0, 2, 3)).astype(f8)

    wout = np.asarray(W_out, np.float32).astype(bf16)
    Wo64 = np.asarray(W_out, np.float64)
    ao = np.asarray(a_out, np.float64)[:, 0]
    wa12 = np.ascontiguousarray(
        np.stack([Wo64 @ ao[:NO], Wo64 @ ao[NO:]], axis=-1)).astype(bf16)

    in_maps = []
    for d in range(NCORES):
        rows = slice(R * d, R * (d + 1))
        xT8own = np.ascontiguousarray(xT8[:, :, rows])
        maskT = np.ascontiguousarray(
            mask_full[rows].T.astype(bf16).reshape(NB, P, R)
            .transpose(1, 0, 2))
        in_maps.append({
            "xT8": xT8, "xT8own": xT8own, "wa1p": wa1p, "wall8": wall8,
            "maskT": maskT, "wout": wout, "wa12": wa12,
        })
    return in_maps


def kernel(x, adj, Ws, As, W_out, a_out, trace=False):
    if "nc" not in _CACHE:
        _CACHE["nc"] = _build()
    nc = _CACHE["nc"]
    in_maps = _prep_inputs(x, adj, Ws, As, W_out, a_out)
    res = run_bass_kernel_spmd(nc, in_maps, list(range(NCORES)), trace=trace)
    out = np.concatenate([res.results[d]["out"] for d in range(NCORES)],
                         axis=0).astype(np.float32)
    if trace:
        kernel.last_exec_time_ns = res.exec_time_ns
    return out


# revision 18
# speedup vs baseline: 1.3013x; 1.0560x over previous
"""GAT (2-layer, 8-head) Bass kernel for 8 Trainium2 NeuronCores.

Strategy (replicated-h, single collective):
  - Core d owns 512 rows (nodes) i in [512d, 512d+512).
  - Layer-1: EVERY core computes h_k = x @ W_k for ALL 4096 nodes and all
    8 heads locally via fp8 DoubleRow matmuls (2 k-subtiles per pass, 2x
    PE rate); s2 = h @ a2 rides the same matmul as 4 extra columns per
    half.  This removes all four layer-1 AllGathers, whose first-start
    floor (~80us from kernel start, regardless of trigger time) made the
    gather pipeline idle every engine for ~50us.
  - Payload stationaries [v_j*h_j | v_j] are built locally per (head,
    j-block) from the staged h via one tensor_scalar each; attention per
    head is the separable-exponential decomposition
        exp(leakyrelu(s1_i + s2_j)) = u_i * v_j * max(w_i * z_j, 1)
    with u = exp(.2 s1) (cancels in softmax), v = exp(.2 s2),
    w = exp(.8 s1), z = exp(.8 s2).  The unnormalized masked score matrix
    (transposed, [j,i]-layout) U[j,i] = max(Wb[j,i]*z_j, 1) * mask[j,i]
    is built with one RELU (ACT, with a PE mask-matmul recovering the
    "+1") or one dual-scalar TENSOR_SCALAR (DVE) per tile, then one
    TENSOR_TENSOR applying the mask per 4-tile group; contracted on the
    PE against [v*h | v] for numerator and denominator in one PSUM pass.
  - w_i (own rows) comes from a tiny dedicated fp8 matmul
    (x_own @ (W@a1)) so Wb_k is ready ~2us in.
  - Layer-2: z = mean_k elu(h'_k) for own rows; ONE small AllGather
    shares the layer-2 payload; phase E as before.  A dummy 1-element
    AllGather issued at t~0 guarantees the CC subsystem is warm.
"""

import numpy as np
import ml_dtypes

import concourse.bass as bass
import concourse.bacc as bacc
import concourse.tile as tile
import concourse.mybir as mybir
from concourse.bass_utils import run_bass_kernel_spmd
from concourse.masks import make_identity

dt = mybir.dt
Alu = mybir.AluOpType
Act = mybir.ActivationFunctionType
DR = mybir.MatmulPerfMode.DoubleRow

NCORES = 8
N, F, NH, KH, NO = 4096, 512, 64, 8, 56
P = 128
R = N // NCORES          # rows per core = 512
IT = R // P              # i-tiles per core = 4
NB = N // P              # j-blocks = 32
GRP = 4                  # j-blocks per TT group
NACT = 2                 # ACT-produced tiles per group
C2 = NO + 2              # layer-2 payload cols
HC = 4 * NH + 4          # phase-A' matmul cols per half: 4 heads + 4 s2
f8 = ml_dtypes.float8_e4m3
bf16 = ml_dtypes.bfloat16

_CACHE: dict = {}


def _build():
    nc = bacc.Bacc("TRN2", target_bir_lowering=False, debug=False,
                   num_devices=NCORES)

    # ---- I/O -----------------------------------------------------------
    xT8_d = nc.dram_tensor("xT8", [P, 4, N], dt.float8e4,
                           kind="ExternalInput")
    xT8own_d = nc.dram_tensor("xT8own", [P, 4, R], dt.float8e4,
                              kind="ExternalInput")
    wa1p_d = nc.dram_tensor("wa1p", [P, 4, 2, 97], dt.float8e4,
                            kind="ExternalInput")
    wall8_d = nc.dram_tensor("wall8", [P, 4, 2, HC], dt.float8e4,
                             kind="ExternalInput")
    maskT_d = nc.dram_tensor("maskT", [P, NB, R], dt.bfloat16,
                             kind="ExternalInput")
    wout_d = nc.dram_tensor("wout", [NH, NO], dt.bfloat16,
                            kind="ExternalInput")
    wa12_d = nc.dram_tensor("wa12", [NH, 2], dt.bfloat16,
                            kind="ExternalInput")
    out_d = nc.dram_tensor("out", [R, NO], dt.float32, kind="ExternalOutput")

    with tile.TileContext(nc) as tc:
        _emit(nc, tc, xT8_d, xT8own_d, wa1p_d, wall8_d, maskT_d, wout_d,
              wa12_d, out_d)

    nc.compile()
    return nc


def _emit(nc, tc, xT8_d, xT8own_d, wa1p_d, wall8_d, maskT_d, wout_d,
          wa12_d, out_d):
    from contextlib import ExitStack
    ctx = ExitStack()
    with ctx:
        const = ctx.enter_context(tc.tile_pool(name="const", bufs=1))
        dram = ctx.enter_context(tc.tile_pool(name="dram", bufs=1,
                                              space="DRAM"))
        pa = ctx.enter_context(tc.tile_pool(name="pa", bufs=1, space="PSUM"))
        pb = ctx.enter_context(tc.tile_pool(name="pb", bufs=2, space="PSUM"))
        pt = ctx.enter_context(tc.tile_pool(name="pt", bufs=1, space="PSUM"))
        sp = ctx.enter_context(tc.tile_pool(name="sp", bufs=3))
        hp = ctx.enter_context(tc.tile_pool(name="hp", bufs=2))
        pp2 = ctx.enter_context(tc.tile_pool(name="pp2", bufs=4))
        tp = ctx.enter_context(tc.tile_pool(name="tp", bufs=3))
        up = ctx.enter_context(tc.tile_pool(name="up", bufs=4))
        bp = ctx.enter_context(tc.tile_pool(name="bp", bufs=1))
        cp = ctx.enter_context(tc.tile_pool(name="cp", bufs=4))
        zp = ctx.enter_context(tc.tile_pool(name="zp", bufs=1))

        # ---- CC warm-up: issued first so ncfw init (~80us, runs on its
        # own clock) overlaps phase A'/B and the layer-2 gather is warm.
        warm_in = dram.tile([1, 16], dt.bfloat16, name="warm_in")
        warm_out = dram.tile([NCORES, 16], dt.bfloat16, addr_space="Shared",
                             name="warm_out")
        nc.gpsimd.collective_compute(
            "AllGather", Alu.bypass,
            ins=[warm_in.opt()], outs=[warm_out.opt()],
            replica_groups=[list(range(NCORES))])

        # ---- resident loads ------------------------------------------
        wall8_sb = const.tile([P, 4, 2, HC], dt.float8e4)
        nc.sync.dma_start(out=wall8_sb, in_=wall8_d.ap())
        wa1p_sb = const.tile([P, 4, 2, 97], dt.float8e4)
        nc.sync.dma_start(out=wa1p_sb, in_=wa1p_d.ap())
        xT8own_sb = const.tile([P, 4, R], dt.float8e4)
        nc.sync.dma_start(out=xT8own_sb, in_=xT8own_d.ap())
        wout_sb = const.tile([NH, NO], dt.bfloat16)
        nc.sync.dma_start(out=wout_sb, in_=wout_d.ap())
        wa12_sb = const.tile([NH, 2], dt.bfloat16)
        nc.sync.dma_start(out=wa12_sb, in_=wa12_d.ap())
        # x^T for all nodes, in 8 column chunks so jb0 unblocks early;
        # mask chunks interleaved on the gpsimd queue.
        xT8_sb = const.tile([P, 4, N], dt.float8e4)
        mask_sb = const.tile([P, NB, R], dt.bfloat16)

        def _x_chunk(c):
            lo, hi = R * c, R * (c + 1)
            nc.sync.dma_start(out=xT8_sb[:, :, lo:hi],
                              in_=xT8_d.ap()[:, :, lo:hi])

        def _mask_chunk(c):
            lo, hi = 4 * c, 4 * (c + 1)
            nc.gpsimd.dma_start(out=mask_sb[:, lo:hi, :],
                                in_=maskT_d.ap()[:, lo:hi, :])

        _x_chunk(0)
        _x_chunk(1)
        _mask_chunk(0)
        for c in range(2, 8):
            _x_chunk(c)
            _mask_chunk(c - 1)
        _mask_chunk(7)

        idb = const.tile([P, P], dt.bfloat16)
        make_identity(nc, idb)
        idf = const.tile([P, P], dt.float32)
        make_identity(nc, idf)
        neg1 = const.tile([P, 1], dt.float32)
        nc.vector.memset(neg1, -1.0)
        ln8 = const.tile([P, 1], dt.float32)
        nc.vector.memset(ln8, float(np.log(0.125)))
        ones1 = const.tile([1, P], dt.bfloat16)
        nc.vector.memset(ones1, 1.0)

        # persistent layer-1 tensors
        staging = zp.tile([P, NB, 2, HC], dt.bfloat16, name="staging")
        pay_v = zp.tile([P, NB, 2, 4, NH + 1], dt.bfloat16, name="pay_v")
        z_all = zp.tile([P, NB, 2, 4], dt.float32, name="z_all")
        v_all = zp.tile([P, NB, 2, 4], dt.float32, name="v_all")
        u_all = zp.tile([P, IT, KH, NH], dt.float32)   # h'_k per head

        ag2_ins = [dram.tile([P, 2, C2], dt.bfloat16, name=f"ag2i{h}")
                   for h in range(2)]
        ag2_outs = [dram.tile([NCORES * P, 2, C2], dt.bfloat16,
                              addr_space="Shared", name=f"ag2o{h}")
                    for h in range(2)]

        # ---- w rows for own i (tiny fp8 matmuls, ready ~2us); s1 rows
        # packed at partition stride 32 so the ACT exp reads are
        # base-partition aligned.
        wrows = []
        for t in range(2):
            ps_s4 = pa.tile([P, R], dt.float32, tag="wb", bufs=1,
                            name=f"ps_s4{t}")
            for fb in range(4):
                nc.tensor.matmul(ps_s4[0:97, :], wa1p_sb[:, fb, t, :],
                                 xT8own_sb[:, fb, :],
                                 start=(fb == 0), stop=(fb == 3))
            for j in range(4):
                wr = sp.tile([1, R], dt.bfloat16, tag="wrow8", bufs=8,
                             name=f"wrow{4 * t + j}")
                nc.scalar.activation(wr, ps_s4[32 * j:32 * j + 1, :],
                                     Act.Exp, scale=0.8)
                wrows.append(wr)
        Wb_k = []
        for k in range(KH):
            ps_wb = pa.tile([P, R], dt.float32, tag="wb", bufs=1)
            nc.tensor.matmul(ps_wb, ones1, wrows[k],
                             start=True, stop=True)
            Wb = sp.tile([P, R], dt.bfloat16, tag="Wb", bufs=8,
                         name=f"Wb{k}")
            nc.scalar.activation(Wb, ps_wb, Act.Copy)
            Wb_k.append(Wb)

        # ================= phase A': replicated h + payload ===========
        for jb in range(NB):
            js = slice(jb * P, (jb + 1) * P)
            for hh in range(2):
                ps_h = pa.tile([P, HC], dt.float32, tag="hn", bufs=2,
                               name=f"ps_h{hh}")
                for pr in range(2):
                    nc.tensor.matmul(ps_h,
                                     xT8_sb[:, 2 * pr:2 * pr + 2, js],
                                     wall8_sb[:, 2 * pr:2 * pr + 2, hh, :],
                                     start=(pr == 0), stop=(pr == 1),
                                     perf_mode=DR)
                nc.scalar.activation(staging[:, jb, hh, :], ps_h,
                                     Act.Copy)
            if jb % 4 == 3:
                g = slice(jb - 3, jb + 1)
                nc.scalar.activation(z_all[:, g, :, :],
                                     staging[:, g, :, 4 * NH:],
                                     Act.Exp, scale=0.8)
                nc.scalar.activation(v_all[:, g, :, :],
                                     staging[:, g, :, 4 * NH:],
                                     Act.Exp, scale=0.2)
                nc.vector.tensor_copy(pay_v[:, g, :, :, NH],
                                      v_all[:, g, :, :])
                for jb2 in range(jb - 3, jb + 1):
                    for k in range(KH):
                        hh, q = k // 4, k % 4
                        nc.vector.tensor_scalar(
                            pay_v[:, jb2, hh, q, 0:NH],
                            staging[:, jb2, hh, q * NH:(q + 1) * NH],
                            v_all[:, jb2, hh, q:q + 1], None, Alu.mult)

        # ================= phase B: per-head attention ================
        def attn_units(Wb, ps_nm, triples, grp, nact, first, last, ttag):
            """triples: (stationary_ap, z_scalar_ap, jb) in order; grp
            consecutive-jb blocks share one mask TT; first nact of each
            group are ACT-produced (relu(z*w-1), '+1' recovered by an
            extra mask matmul)."""
            n = len(triples)
            for g0 in range(0, n, grp):
                seg = triples[g0:g0 + grp]
                Tg = tp.tile([P, grp, R], dt.bfloat16, tag=ttag)
                for q, (stat, zsc, jb) in enumerate(seg):
                    if q < nact:
                        nc.scalar.activation(Tg[:, q, :], Wb, Act.Relu,
                                             bias=neg1[:, 0:1], scale=zsc)
                    else:
                        nc.vector.tensor_scalar(Tg[:, q, :], Wb, zsc, 1.0,
                                                Alu.mult, Alu.max)
                jb0 = seg[0][2]
                Ug = up.tile([P, grp, R], dt.bfloat16, tag=ttag + "u")
                nc.vector.tensor_tensor(Ug, Tg,
                                        mask_sb[:, jb0:jb0 + grp, :],
                                        Alu.mult)
                for q, (stat, zsc, jb) in enumerate(seg):
                    st = first and g0 == 0 and q == 0
                    sp_ = last and g0 + grp >= n and q == len(seg) - 1
                    nc.tensor.matmul(ps_nm, stat, Ug[:, q, :],
                                     start=st, stop=sp_)
                    if q < nact:
                        nc.tensor.matmul(ps_nm, stat, mask_sb[:, jb, :],
                                         start=False, stop=False)

        def finish_unit(ps_nm, ncols):
            nmf = sp.tile([ncols, R], dt.float32, tag="nmf")
            nc.any.tensor_copy(nmf, ps_nm)
            ps_t = pt.tile([P, IT, ncols], dt.float32, tag="tr")
            for isl in range(IT):
                sl = slice(isl * P, (isl + 1) * P)
                nc.tensor.transpose(ps_t[:, isl, :], nmf[:, sl],
                                    idf[0:ncols, 0:ncols])
            return ps_t

        ps_misc = pa.tile([P, IT, NH], dt.float32, tag="sall")
        ps_zsum = ps_misc
        for k in range(KH):
            ps_nm = pb.tile([NH + 1, R], dt.float32, tag="nm")
            triples = [(pay_v[:, jb, k // 4, k % 4, :],
                        z_all[:, jb, k // 4, k % 4:k % 4 + 1], jb)
                       for jb in range(NB)]
            attn_units(Wb_k[k], ps_nm, triples, GRP, NACT, True, True, "T")
            ps_t = finish_unit(ps_nm, NH + 1)
            rc4 = sp.tile([P, IT], dt.float32, tag="rc")
            nc.vector.reciprocal(rc4, ps_t[:, :, NH])
            for isl in range(IT):
                nc.scalar.activation(u_all[:, isl, k, :],
                                     ps_t[:, isl, 0:NH], Act.Copy,
                                     scale=rc4[:, isl:isl + 1])
            rn = cp.tile([P, IT, NH], dt.float32, tag="rn")
            nc.scalar.activation(rn, u_all[:, :, k, :], Act.Relu,
                                 scale=-1.0)
            Bpp = cp.tile([P, IT, NH], dt.bfloat16, tag="Bp")
            nc.scalar.activation(Bpp, rn, Act.Exp, bias=ln8[:, 0:1],
                                 scale=-1.0)
            Dpp = cp.tile([P, IT, NH], dt.bfloat16, tag="Dp")
            nc.scalar.activation(Dpp, u_all[:, :, k, :], Act.Relu,
                                 scale=0.125)
            nc.tensor.matmul(ps_zsum, idb, Dpp,
                             start=(k == 0), stop=False)
            nc.tensor.matmul(ps_zsum, idb, Bpp,
                             start=False, stop=(k == KH - 1))

        # ================= phase D: layer-2 h2 + payload (2 halves) =====
        zbf = zp.tile([P, IT, NH], dt.bfloat16)
        ps_zT = pa.tile([NH, R], dt.bfloat16, tag="wb", bufs=1)
        zT = hp.tile([NH, R], dt.bfloat16, tag="hT_s")
        h2T = hp.tile([NO, R], dt.bfloat16, tag="h2T_s")
        s2row = sp.tile([2, R], dt.bfloat16, tag="srow_s", bufs=1)
        w2row = sp.tile([1, R], dt.bfloat16, tag="w2row", bufs=1)
        ps_wb2 = pa.tile([P, R], dt.float32, tag="hn", bufs=2)
        Wb2 = sp.tile([P, R], dt.bfloat16, tag="Wb2", bufs=1)
        ps_h2n = pa.tile([P, IT, C2], dt.bfloat16, tag="hn", bufs=2)
        v2col = sp.tile([P, IT], dt.float32, tag="v2col", bufs=1)
        for h in range(2):
            hs = slice(h * 256, (h + 1) * 256)
            nc.vector.tensor_scalar(zbf[:, 2 * h:2 * h + 2, :],
                                    ps_zsum[:, 2 * h:2 * h + 2, :],
                                    -1.0, None, Alu.add)
            for isl in (2 * h, 2 * h + 1):
                sl = slice(isl * P, (isl + 1) * P)
                nc.tensor.transpose(ps_zT[:, sl], zbf[:, isl, :], idb)
            nc.vector.tensor_copy(zT[:, hs], ps_zT[:, hs])
            ps_h2T = pa.tile([NO, 256], dt.float32, tag="srow",
                             name=f"ps_h2T{h}")
            nc.tensor.matmul(ps_h2T, wout_sb, zT[:, hs],
                             start=True, stop=True)
            nc.any.tensor_copy(h2T[:, hs], ps_h2T)
            ps_s12 = pa.tile([2, 256], dt.float32, tag="srow",
                             name=f"ps_s12{h}")
            nc.tensor.matmul(ps_s12, wa12_sb, zT[:, hs],
                             start=True, stop=True)
            nc.any.tensor_copy(s2row[:, hs], ps_s12)
            nc.scalar.activation(w2row[:, hs], s2row[0:1, hs], Act.Exp,
                                 scale=0.8)
            nc.tensor.matmul(ps_wb2[:, hs], ones1, w2row[:, hs],
                             start=True, stop=True)
            nc.vector.tensor_copy(Wb2[:, hs], ps_wb2[:, hs])
            pay2 = pp2.tile([P, 2, C2], dt.bfloat16, tag="pay2")
            for ii, isl in enumerate((2 * h, 2 * h + 1)):
                sl = slice(isl * P, (isl + 1) * P)
                nc.tensor.transpose(ps_h2n[:, isl, 0:NO], h2T[:, sl],
                                    idb[0:NO, 0:NO])
                nc.tensor.transpose(ps_h2n[:, isl, NO:NO + 2], s2row[:, sl],
                                    idb[0:2, 0:2])
            nc.scalar.activation(v2col[:, 2 * h:2 * h + 2],
                                 ps_h2n[:, 2 * h:2 * h + 2, NO + 1],
                                 Act.Exp, scale=0.2)
            for ii, isl in enumerate((2 * h, 2 * h + 1)):
                nc.vector.tensor_scalar(pay2[:, ii, 0:NO],
                                        ps_h2n[:, isl, 0:NO],
                                        v2col[:, isl:isl + 1], None,
                                        Alu.mult)
            nc.vector.tensor_copy(pay2[:, :, NO],
                                  v2col[:, 2 * h:2 * h + 2])
            nc.scalar.activation(pay2[:, :, NO + 1],
                                 ps_h2n[:, 2 * h:2 * h + 2, NO + 1],
                                 Act.Exp, scale=0.8)
            nc.sync.dma_start(out=ag2_ins[h], in_=pay2)
            nc.gpsimd.collective_compute(
                "AllGather", Alu.bypass,
                ins=[ag2_ins[h].opt()], outs=[ag2_outs[h].opt()],
                replica_groups=[list(range(NCORES))])

        # ================= phase E: attention-2 + softmax ===============
        zf2 = sp.tile([P, NCORES, IT], dt.float32, tag="zf")
        ps_nm2 = pb.tile([NO + 1, R], dt.float32, tag="nm")
        hsb2 = bp.tile([P, NCORES, 2, 2, C2], dt.bfloat16, name="hsb2")
        for hg in range(2):
            nc.sync.dma_start(
                out=hsb2[:, :, hg, :, :],
                in_=ag2_outs[hg].rearrange(
                    "(core p) i c -> p core i c", p=P))
            nc.vector.tensor_copy(
                zf2[:, :, 2 * hg:2 * hg + 2],
                hsb2[:, :, hg, :, NO + 1])
        trips = [(hsb2[:, jb // IT, (jb % IT) // 2, (jb % IT) % 2,
                       0:NO + 1],
                  zf2[:, jb // IT, jb % IT:jb % IT + 1], jb)
                 for jb in range(NB)]
        order = [jb for jb in range(NB) if jb % IT < 2] + \
                [jb for jb in range(NB) if jb % IT >= 2]
        for gi in range(0, NB, 2):
            j0, j1 = order[gi], order[gi + 1]
            attn_units(Wb2, ps_nm2, [trips[j0], trips[j1]], 2, 1,
                       gi == 0, gi + 2 >= NB, "T")
        ps_t2 = finish_unit(ps_nm2, NO + 1)
        for isl in range(IT):
            rc = sp.tile([P, 1], dt.float32, tag="rc")
            nc.vector.reciprocal(rc, ps_t2[:, isl, NO:NO + 1])
            ue = cp.tile([P, NO], dt.float32, tag="ue")
            nc.vector.tensor_scalar(ue, ps_t2[:, isl, 0:NO], rc, None,
                                    Alu.mult)
            e2 = cp.tile([P, NO], dt.float32, tag="e2")
            nc.scalar.activation(e2, ue, Act.Exp)
            t1 = cp.tile([P, NO], dt.float32, tag="t1")
            nc.vector.tensor_scalar(t1, e2, 1.0, -1.0, Alu.min, Alu.add)
            el = cp.tile([P, NO], dt.float32, tag="el")
            nc.vector.scalar_tensor_tensor(el, ue, 0.0, t1, Alu.max, Alu.add)
            # |elu(h2')| is O(1): plain exp is overflow-safe, and the
            # max-subtraction cancels exactly in the softmax ratio
            ex = cp.tile([P, NO], dt.float32, tag="ex")
            sm = sp.tile([P, 1], dt.float32, tag="sm")
            nc.scalar.activation(ex, el, Act.Exp, accum_out=sm)
            rc2 = sp.tile([P, 1], dt.float32, tag="rc2")
            nc.vector.reciprocal(rc2, sm)
            oo = cp.tile([P, NO], dt.float32, tag="oo")
            nc.vector.tensor_scalar(oo, ex, rc2, None, Alu.mult)
            nc.sync.dma_start(out=out_d.ap()[isl * P:(isl + 1) * P, :],
                              in_=oo)


def _prep_inputs(x, adj, Ws, As, W_out, a_out):
    x64 = np.asarray(x, np.float64)
    adj_np = np.asarray(adj)
    mask_full = adj_np > 0
    Ws64 = np.asarray(Ws, np.float64)               # [8, 512, 64]
    As64 = np.asarray(As, np.float64)[:, :, 0]      # [8, 128]

    xT8 = np.ascontiguousarray(
        x64.T.reshape(4, P, N).transpose(1, 0, 2)).astype(f8)

    # wall8: per half hh, cols [h_{4 heads} (256) | s2_{4 heads} (4)]
    wall = np.zeros((F, 2, HC), np.float64)
    wa1 = np.zeros((F, 2, 97), np.float64)
    for k in range(KH):
        hh, q = k // 4, k % 4
        wall[:, hh, q * NH:(q + 1) * NH] = Ws64[k]
        wall[:, hh, 4 * NH + q] = Ws64[k] @ As64[k, NH:]
        wa1[:, hh, 32 * q] = Ws64[k] @ As64[k, :NH]
    wall8 = np.ascontiguousarray(
        wall.reshape(4, P, 2, HC).transpose(1, 0, 2, 3)).astype(f8)
    wa1p = np.ascontiguousarray(
        wa1.reshape(4, P, 2, 97).transpose(1, 0, 2, 3)).astype(f8)

    wout = np.asarray(W_out, np.float32).astype(bf16)
    Wo64 = np.asarray(W_out, np.float64)
    ao = np.asarray(a_out, np.float64)[:, 0]
    wa12 = np.ascontiguousarray(
        np.stack([Wo64 @ ao[:NO], Wo64 @ ao[NO:]], axis=-1)).astype(bf16)

    in_maps = []
    for d in range(NCORES):
        rows = slice(R * d, R * (d + 1))
        xT8own = np.ascontiguousarray(xT8[:, :, rows])
        maskT = np.ascontiguousarray(
            mask_full[rows].T.astype(bf16).reshape(NB, P, R)
            .transpose(1, 0, 2))
        in_maps.append({
            "xT8": xT8, "xT8own": xT8own, "wa1p": wa1p, "wall8": wall8,
            "maskT": maskT, "wout": wout, "wa12": wa12,
        })
    return in_maps


def kernel(x, adj, Ws, As, W_out, a_out, trace=False):
    if "nc" not in _CACHE:
        _CACHE["nc"] = _build()
    nc = _CACHE["nc"]
    in_maps = _prep_inputs(x, adj, Ws, As, W_out, a_out)
    res = run_bass_kernel_spmd(nc, in_maps, list(range(NCORES)), trace=trace)
    out = np.concatenate([res.results[d]["out"] for d in range(NCORES)],
                         axis=0).astype(np.float32)
    if trace:
        kernel.last_exec_time_ns = res.exec_time_ns
    return out
